# revision 14
# baseline (speedup 1.0000x reference)
"""Trainium2 Bass kernel for a recurrent adaptive-LIF SNN.

Network (per reference):
    B=1024, T=100, n_in=120, h1=512, h2=256, n_out=35
    per step t:
        cur1 = x_t @ W1.T + s1 @ Wrec.T
        a1' = rho1*a1 + (1-rho1)*s1
        v1' = alpha1*v1*(1-s1) + (1-alpha1)*cur1
        s1' = (v1' - (1 + beta_a1*a1') > 0)
        cur2 = s1' @ W2.T ; same LIF for layer 2
        vo' = beta_out*vo + (1-beta_out)*(s2' @ W3.T)
    out = mean_t vo(t)

Sharding: data-parallel over batch across 8 cores (128 batch/core),
weights replicated; the sequential T loop is local per core.

Layout: feature-major [feature -> partitions, batch -> free].

Layer 1 (shifted P1 := v1'-1; scaled copy cp1 := P1/cb1, cb1=beta_a1(1-rho1)):
    p1psum = W1a@[x;1] + WrecF@s1 + a1*cb1*(cp1_prev - r1_prev)   (diag matmuls)
    cp1    = ACT(p1psum, scale=1/cb1)        [= P1/cb1]
    u1'    = rho1*u1 + s1                    [STT]
    s1'    = (u1' < cp1)                     [TT is_lt  <=> cb1*u1' < P1]
    r1'    = s1' * cp1                       [TT mult, GPSIMD]
  since -a1*q1 = -a1*(s1-1)*P1 = a1*cb1*(cp1 - r1).  W1a has a const row
  (a1-1); WrecF = ((1-a1)Wrec).T - a1*I runs as fp8e4 DoubleRow with a 2^7
  weight scale balanced by fp8 spike mirrors sf = 2^-7*s1.
  Init: cp1_init = -1/cb1, r1_init = 0  (reproduces q1_init = 1).

Layer 2 (scaled Pt2 := v2/cb2; psum tracks Pt2 - a2/cb2):
    p2psum = W2s@s1' + a2*(cp2_prev - r2_prev) - (a2/cb2)*s2_prev  (diags)
    cp2    = ACT(p2psum, bias=(a2-1)/cb2)    [= Pt2 - 1/cb2]
    W2t'   = rho2*W2t + s2                   [STT]
    s2'    = (W2t' < cp2)                    [TT  <=> W2t'+1/cb2 < Pt2]
    r2'    = s2' * cp2                       [TT mult, GPSIMD]
  W2s = ((1-a2)/cb2 * W2).T.  Init r2_init = 1/cb2, cp2_init = 0.

Output (closed form, no integrator state):
    out = sum_t c_t * (W3 @ s2(t)),  c_t = (1 - beta_out^(T-t))/T
    via S_ps += I@(c_t*s2(t)) on the PE; W3 applied once at the end.

Engine split per step: PE matmuls (w1 prefetch, fp8-DR Wrec, bf16 W2, cheap
diag compensations, output accumulation); ACT scaled PSUM->SBUF copies +
s2c; DVE u1/W2t STT, spike compares, fp8 mirrors; GPSIMD the r = s*cp
reset products (consumed by late diag matmuls to hide Q7 latency).
"""

import sys
import numpy as np

sys.path.insert(0, "/opt/trn_rl_repo")

import ml_dtypes

bf16 = ml_dtypes.bfloat16
f8e4 = ml_dtypes.float8_e4m3

# Problem constants (hardcoded per contract)
B, T, N_IN, H1, H2, N_OUT = 1024, 100, 120, 512, 256, 35
N_CORES = 8
BC = B // N_CORES  # 128 batch per core
C1 = H1 // 128     # 4 feature chunks, layer 1
C2 = H2 // 128     # 2 feature chunks, layer 2
K1 = N_IN + 1      # x augmented with a constant-one row

USE_DR = True      # fp8e4 DoubleRow for the Wrec matmuls
FP8_SHIFT = 7      # wrec8 = 2^7 * WrecF, fp8 spike mirror value = 2^-7

_CACHE = {}


def _build(alpha1, rho1, beta_a1, alpha2, rho2, beta_a2, beta_out):
    import concourse.bacc as bacc
    import concourse.mybir as mybir
    import concourse.tile as tile
    from concourse.alu_op_type import AluOpType

    fp32 = mybir.dt.float32
    bft = mybir.dt.bfloat16
    f8t = mybir.dt.float8e4
    A = AluOpType
    IDENT = mybir.ActivationFunctionType.Identity
    DR = mybir.MatmulPerfMode.DoubleRow

    a1 = float(alpha1)
    a2 = float(alpha2)
    cb1 = float(beta_a1 * (1.0 - rho1))
    cb2 = float(beta_a2 * (1.0 - rho2))
    sfp8 = float(2.0 ** -FP8_SHIFT)

    nc = bacc.Bacc()

    x_d = nc.declare_dram_parameter("x", [K1, T, BC], bft, isOutput=False)
    w1_d = nc.declare_dram_parameter("w1s", [K1, C1, 128], bft, isOutput=False)
    if USE_DR:
        wr_d = nc.declare_dram_parameter(
            "wrec8", [128, 2, C1, 2, 128], f8t, isOutput=False
        )
    else:
        wr_d = nc.declare_dram_parameter(
            "wrecs", [128, C1, C1, 128], bft, isOutput=False
        )
    w2_d = nc.declare_dram_parameter("w2s", [128, C1, C2, 128], bft, isOutput=False)
    w3_d = nc.declare_dram_parameter("w3s", [128, C2, N_OUT], bft, isOutput=False)
    # five diagonal compensation matrices + identity
    dg_d = nc.declare_dram_parameter("diags", [128, 6, 128], bft, isOutput=False)
    out_d = nc.declare_dram_parameter("out", [N_OUT, BC], fp32, isOutput=True)

    XCH = 10  # x preload chunks
    TP = T // XCH

    # per-step output weights c_t = (1 - beta^(T-t))/T
    cw = [(1.0 - float(beta_out) ** (T - t)) / T for t in range(T)]

    with tile.TileContext(nc) as tc:
        with (
            tc.tile_pool(name="wpool", bufs=1) as wpool,
            tc.tile_pool(name="xpool", bufs=1) as xpool,
            tc.tile_pool(name="st1", bufs=3) as st1,
            tc.tile_pool(name="st2", bufs=3) as st2,
            tc.tile_pool(name="cp", bufs=3) as cpp,
            tc.tile_pool(name="sf", bufs=3) as sfp,
            tc.tile_pool(name="ps1a", bufs=2, space="PSUM") as ps1a,
            tc.tile_pool(name="ps1b", bufs=2, space="PSUM") as ps1b,
            tc.tile_pool(name="ps2", bufs=2, space="PSUM") as ps2,
            tc.tile_pool(name="psS", bufs=1, space="PSUM") as psS,
            tc.tile_pool(name="psO", bufs=1, space="PSUM") as psO,
        ):
            # ---- resident weights ----
            w1_s = wpool.tile([K1, C1, 128], bft, tag="w1")
            nc.sync.dma_start(w1_s[:], w1_d[:])
            if USE_DR:
                wr_s = wpool.tile([128, 2, C1, 2, 128], f8t, tag="wr")
            else:
                wr_s = wpool.tile([128, C1, C1, 128], bft, tag="wr")
            nc.sync.dma_start(wr_s[:], wr_d[:])
            w2_s = wpool.tile([128, C1, C2, 128], bft, tag="w2")
            nc.sync.dma_start(w2_s[:], w2_d[:])
            w3_s = wpool.tile([128, C2, N_OUT], bft, tag="w3")
            nc.sync.dma_start(w3_s[:], w3_d[:])
            dg_s = wpool.tile([128, 6, 128], bft, tag="dg")
            nc.sync.dma_start(dg_s[:], dg_d[:])
            # diag slots: 0:+a1*cb1  1:-a1*cb1  2:+a2  3:-a2  4:-a2/cb2  5:I
            PD1, ND1, PD2, ND2, SD2, ID = (dg_s[:, i, :] for i in range(6))
            # per-partition bias column for the layer-2 scaled copy
            b2_s = wpool.tile([128, 1], fp32, tag="b2")
            nc.vector.memset(b2_s[:], (a2 - 1.0) / cb2)

            # ---- x preload in chunks ----
            x_tiles = []
            for i in range(XCH):
                xt = xpool.tile([K1, TP, BC], bft, tag=f"x{i}")
                nc.sync.dma_start(xt[:], x_d[:, i * TP : (i + 1) * TP, :])
                x_tiles.append(xt)

            # ---- initial states ----
            s1 = st1.tile([128, C1, BC], bft, tag="s1")
            nc.vector.memset(s1[:], 0.0)
            u1 = st1.tile([128, C1, BC], bft, tag="u1")
            nc.vector.memset(u1[:], 0.0)
            r1 = st1.tile([128, C1, BC], bft, tag="r1")
            nc.vector.memset(r1[:], 0.0)
            cpa = cpp.tile([128, 2, BC], bft, tag="cpa")
            nc.vector.memset(cpa[:], -1.0 / cb1)
            cpb = cpp.tile([128, 2, BC], bft, tag="cpb")
            nc.vector.memset(cpb[:], -1.0 / cb1)
            sfa = sfp.tile([128, 2, BC], f8t, tag="sfa")
            nc.vector.memset(sfa[:], 0.0)
            sfb = sfp.tile([128, 2, BC], f8t, tag="sfb")
            nc.vector.memset(sfb[:], 0.0)
            w2t = st2.tile([128, C2, BC], bft, tag="w2t")
            nc.vector.memset(w2t[:], 0.0)
            s2 = st2.tile([128, C2, BC], bft, tag="s2")
            nc.vector.memset(s2[:], 0.0)
            r2 = st2.tile([128, C2, BC], bft, tag="r2")
            nc.vector.memset(r2[:], 1.0 / cb2)
            cp2 = cpp.tile([128, C2, BC], bft, tag="cp2")
            nc.vector.memset(cp2[:], 0.0)
            s2c = None

            S_ps = psS.tile([128, C2, BC], fp32, tag="S")

            # ---- prologue: open p1 region-0 groups for t=0 (chunks 0, 2).
            # PSUM start=True clears has_written for the WHOLE bank, so within
            # a bank the two chunk regions must run strictly sequentially:
            # region 1's start fires only after region 0 stops.
            p1a = ps1a.tile([128, 2, BC], fp32, tag="p1a")
            p1b = ps1b.tile([128, 2, BC], fp32, tag="p1b")
            for m in (0, 2):
                ph = p1a if m == 0 else p1b
                nc.tensor.matmul(
                    ph[:, 0, :], w1_s[:, m, :], x_tiles[0][:, 0, :],
                    start=True, stop=False,
                )

            def l1_chunk(m, ph, mh):
                # wrec + diag compensations for output chunk m into region mh
                o = ph[:, mh, :]
                if USE_DR:
                    nc.tensor.matmul(o, wr_s[:, 0, m, :, :], sfa[:],
                                     start=False, stop=False, perf_mode=DR)
                    nc.tensor.matmul(o, wr_s[:, 1, m, :, :], sfb[:],
                                     start=False, stop=False, perf_mode=DR)
                else:
                    for k in range(C1):
                        nc.tensor.matmul(o, wr_s[:, k, m, :], s1[:, k, :],
                                         start=False, stop=False)
                cph = cpa if m < 2 else cpb
                nc.tensor.matmul(o, PD1, cph[:, m % 2, :], start=False, stop=False)
                rh = r1[:, m, :]
                nc.tensor.matmul(o, ND1, rh, start=False, stop=True)

            for t in range(T):
                # ---- finish p1(t): strictly sequential regions per bank ----
                xsl_t = x_tiles[t // TP][:, t % TP, :]
                l1_chunk(0, p1a, 0)
                l1_chunk(2, p1b, 0)
                nc.tensor.matmul(p1a[:, 1, :], w1_s[:, 1, :], xsl_t,
                                 start=True, stop=False)
                nc.tensor.matmul(p1b[:, 1, :], w1_s[:, 3, :], xsl_t,
                                 start=True, stop=False)
                l1_chunk(1, p1a, 1)
                l1_chunk(3, p1b, 1)

                # ---- output accumulation for step t-1 ----
                if t >= 1:
                    for k in range(C2):
                        nc.tensor.matmul(
                            S_ps[:, k, :], ID, s2c[:, k, :],
                            start=(t == 1 and k == 0), stop=False,
                            skip_group_check=True,
                        )

                # ---- scaled PSUM -> SBUF copies of P1 ----
                cpa_n = cpp.tile([128, 2, BC], bft, tag="cpa")
                nc.scalar.activation(cpa_n[:], p1a[:], IDENT, scale=1.0 / cb1)
                cpb_n = cpp.tile([128, 2, BC], bft, tag="cpb")
                nc.scalar.activation(cpb_n[:], p1b[:], IDENT, scale=1.0 / cb1)

                # ---- layer-1 state updates ----
                u1n = st1.tile([128, C1, BC], bft, tag="u1")
                nc.vector.scalar_tensor_tensor(
                    u1n[:], u1[:], float(rho1), s1[:], A.mult, A.add
                )
                s1n = st1.tile([128, C1, BC], bft, tag="s1")
                nc.vector.tensor_tensor(
                    s1n[:, 0:2, :], u1n[:, 0:2, :], cpa_n[:], A.is_lt
                )
                sfa_n = sfp.tile([128, 2, BC], f8t, tag="sfa")
                nc.vector.tensor_scalar(sfa_n[:], s1n[:, 0:2, :], sfp8, None, A.mult)
                nc.vector.tensor_tensor(
                    s1n[:, 2:4, :], u1n[:, 2:4, :], cpb_n[:], A.is_lt
                )
                sfb_n = sfp.tile([128, 2, BC], f8t, tag="sfb")
                nc.vector.tensor_scalar(sfb_n[:], s1n[:, 2:4, :], sfp8, None, A.mult)

                # reset products on GPSIMD (consumed by next step's late diags)
                r1n = st1.tile([128, C1, BC], bft, tag="r1")
                nc.gpsimd.tensor_tensor(
                    r1n[:, 0:2, :], s1n[:, 0:2, :], cpa_n[:], A.mult
                )
                nc.gpsimd.tensor_tensor(
                    r1n[:, 2:4, :], s1n[:, 2:4, :], cpb_n[:], A.mult
                )

                # ---- W1 prefetch into next step's p1 banks (region 0 only;
                # region 1's start must wait for region 0's stop) ----
                if t + 1 < T:
                    p1a_n = ps1a.tile([128, 2, BC], fp32, tag="p1a")
                    p1b_n = ps1b.tile([128, 2, BC], fp32, tag="p1b")
                    xsl = x_tiles[(t + 1) // TP][:, (t + 1) % TP, :]
                    nc.tensor.matmul(p1a_n[:, 0, :], w1_s[:, 0, :], xsl,
                                     start=True, stop=False)
                    nc.tensor.matmul(p1b_n[:, 0, :], w1_s[:, 2, :], xsl,
                                     start=True, stop=False)
                else:
                    p1a_n = p1b_n = None

                # ---- layer 2: p2 = W2s@s1' + diag compensations ----
                # strictly sequential regions within the p2 bank
                p2 = ps2.tile([128, C2, BC], fp32, tag="p2")
                for m in range(C2):
                    o = p2[:, m, :]
                    nc.tensor.matmul(o, ND2, r2[:, m, :], start=True, stop=False)
                    nc.tensor.matmul(o, PD2, cp2[:, m, :], start=False, stop=False)
                    nc.tensor.matmul(o, SD2, s2[:, m, :], start=False, stop=False)
                    nc.tensor.matmul(o, w2_s[:, 0, m, :], s1n[:, 0, :],
                                     start=False, stop=False)
                    nc.tensor.matmul(o, w2_s[:, 1, m, :], s1n[:, 1, :],
                                     start=False, stop=False)
                    nc.tensor.matmul(o, w2_s[:, 2, m, :], s1n[:, 2, :],
                                     start=False, stop=False)
                    nc.tensor.matmul(o, w2_s[:, 3, m, :], s1n[:, 3, :],
                                     start=False, stop=True)

                cp2_n = cpp.tile([128, C2, BC], bft, tag="cp2")
                nc.scalar.activation(cp2_n[:], p2[:], IDENT, bias=b2_s[:])

                # ---- layer-2 state updates ----
                w2tn = st2.tile([128, C2, BC], bft, tag="w2t")
                nc.vector.scalar_tensor_tensor(
                    w2tn[:], w2t[:], float(rho2), s2[:], A.mult, A.add
                )
                s2n = st2.tile([128, C2, BC], bft, tag="s2")
                nc.vector.tensor_tensor(s2n[:], w2tn[:], cp2_n[:], A.is_lt)
                s2cn = st2.tile([128, C2, BC], bft, tag="s2c")
                nc.scalar.activation(s2cn[:], s2n[:], IDENT, scale=cw[t])
                r2n = st2.tile([128, C2, BC], bft, tag="r2")
                nc.gpsimd.tensor_tensor(r2n[:], s2n[:], cp2_n[:], A.mult)

                s1, u1, r1 = s1n, u1n, r1n
                cpa, cpb, sfa, sfb = cpa_n, cpb_n, sfa_n, sfb_n
                w2t, s2, r2, cp2, s2c = w2tn, s2n, r2n, cp2_n, s2cn
                p1a, p1b = p1a_n, p1b_n

            # ---- epilogue: final S term, W3, writeback ----
            for k in range(C2):
                nc.tensor.matmul(
                    S_ps[:, k, :], ID, s2c[:, k, :],
                    start=False, stop=(k == C2 - 1),
                    skip_group_check=True,
                )
            Sb = cpp.tile([128, C2, BC], bft, tag="Sb")
            nc.scalar.activation(Sb[:], S_ps[:], IDENT)
            out_ps = psO.tile([N_OUT, BC], fp32, tag="ops")
            for k in range(C2):
                nc.tensor.matmul(
                    out_ps[:], w3_s[:, k, :], Sb[:, k, :],
                    start=(k == 0), stop=(k == C2 - 1),
                )
            outf = cpp.tile([N_OUT, BC], fp32, tag="outf")
            nc.vector.tensor_scalar(outf[:], out_ps[:], 1.0, None, A.mult)
            nc.sync.dma_start(out_d[:], outf[:])

    nc.compile()
    return nc


def _prep_inputs(x, W1, Wrec, W2, W3, alpha1, rho1, beta_a1, alpha2, rho2, beta_a2, beta_out):
    a1 = float(np.asarray(alpha1).reshape(-1)[0])
    a2 = float(np.asarray(alpha2).reshape(-1)[0])
    r1 = float(np.asarray(rho1).reshape(-1)[0])
    r2 = float(np.asarray(rho2).reshape(-1)[0])
    ba1 = float(np.asarray(beta_a1).reshape(-1)[0])
    ba2 = float(np.asarray(beta_a2).reshape(-1)[0])
    cb1 = ba1 * (1.0 - r1)
    cb2 = ba2 * (1.0 - r2)

    w1s = ((1.0 - np.asarray(alpha1, np.float32)[:, None]) * np.asarray(W1, np.float32)).T
    wrs = ((1.0 - np.asarray(alpha1, np.float32)[:, None]) * np.asarray(Wrec, np.float32)).T
    w2s = (((1.0 - np.asarray(alpha2, np.float32)[:, None]) / cb2) * np.asarray(W2, np.float32)).T
    w3s = np.asarray(W3, np.float32).T

    # layer-1 shift folds:  WrecF = wrs - a1*I ; W1 gains const row (a1-1)
    wrs = wrs - a1 * np.eye(H1, dtype=np.float32)
    w1aug = np.concatenate(
        [w1s, np.full((1, H1), a1 - 1.0, np.float32)], axis=0
    )  # [121, 512]

    w1_a = np.ascontiguousarray(w1aug.reshape(K1, C1, 128)).astype(bf16)
    if USE_DR:
        # wrec8[part, pair, m, i, col] = 2^7 * WrecF[(2*pair+i)*128+part, m*128+col]
        wr4 = wrs.reshape(C1, 128, C1, 128)  # [k, part, m, col]
        wr8 = (2.0 ** FP8_SHIFT) * wr4.reshape(2, 2, 128, C1, 128)
        wr_a = np.ascontiguousarray(wr8.transpose(2, 0, 3, 1, 4)).astype(f8e4)
    else:
        wr_a = np.ascontiguousarray(
            wrs.reshape(C1, 128, C1, 128).transpose(1, 0, 2, 3)
        ).astype(bf16)
    w2_a = np.ascontiguousarray(
        w2s.reshape(C1, 128, C2, 128).transpose(1, 0, 2, 3)
    ).astype(bf16)
    w3_a = np.ascontiguousarray(
        w3s.reshape(C2, 128, N_OUT).transpose(1, 0, 2)
    ).astype(bf16)

    eye = np.eye(128, dtype=np.float32)
    diags = np.stack([
        (a1 * cb1) * eye,     # PD1
        (-a1 * cb1) * eye,    # ND1
        a2 * eye,             # PD2
        (-a2) * eye,          # ND2
        (-a2 / cb2) * eye,    # SD2
        eye,                  # ID
    ], axis=1).astype(bf16)   # [128, 6, 128]

    shared = dict(w1s=w1_a, w2s=w2_a, w3s=w3_a, diags=diags)
    if USE_DR:
        shared["wrec8"] = wr_a
    else:
        shared["wrecs"] = wr_a
    in_maps = []
    for c in range(N_CORES):
        xc = np.asarray(x[c * BC : (c + 1) * BC], np.float32)  # [BC, T, N_IN]
        xfm = xc.transpose(2, 1, 0)  # [N_IN, T, BC]
        xaug = np.concatenate([xfm, np.ones((1, T, BC), np.float32)], axis=0)
        in_maps.append(dict(x=np.ascontiguousarray(xaug).astype(bf16), **shared))
    return in_maps


def kernel(
    x, W1, Wrec, W2, W3,
    alpha1, rho1, beta_a1, alpha2, rho2, beta_a2, beta_out,
    _trace=False,
):
    from concourse.bass_utils import run_bass_kernel_spmd

    key = "nc"
    if key not in _CACHE:
        _CACHE[key] = _build(
            float(np.asarray(alpha1).reshape(-1)[0]),
            float(np.asarray(rho1).reshape(-1)[0]),
            float(np.asarray(beta_a1).reshape(-1)[0]),
            float(np.asarray(alpha2).reshape(-1)[0]),
            float(np.asarray(rho2).reshape(-1)[0]),
            float(np.asarray(beta_a2).reshape(-1)[0]),
            float(np.asarray(beta_out).reshape(-1)[0]),
        )
    nc = _CACHE[key]

    in_maps = _prep_inputs(
        x, W1, Wrec, W2, W3, alpha1, rho1, beta_a1, alpha2, rho2, beta_a2, beta_out
    )
    res = run_bass_kernel_spmd(nc, in_maps, list(range(N_CORES)), trace=_trace)

    out = np.empty((B, N_OUT), np.float32)
    for c in range(N_CORES):
        out[c * BC : (c + 1) * BC] = np.asarray(res.results[c]["out"]).T
    if _trace:
        return out, res
    return out
